# revision 5
# baseline (speedup 1.0000x reference)
"""Single-head attention (q/k/v projections + scores + softmax + PV) on 8 TRN2
NeuronCores.

Sharding: core c handles batch b = c // 2 and query-half h = c % 2, i.e. 2048
queries against that batch's full 4096 keys.  Each core computes its own K/V
projections from the full x_k/x_v of its batch (duplicated across the pair of
cores sharing a batch).

Device-side layout strategy (per core):
  - x tiles are DMA'd naturally ([s, h] rows on partitions) and transposed
    on the PE (fp32 transpose-mode matmul with an identity) so the hidden dim
    lands on partitions for the projection matmuls.
  - Projections are computed directly in transposed form qT/kT/vT [64, S]
    (d on partitions) which is exactly the operand layout the scores matmuls
    need.  All matmuls use float32r (reduced-precision fp32 multiply path,
    full fp32 accumulate): ~1.6e-4 max rel error, 4x faster than true fp32.
  - scores S = qT.T @ kT per 128-query tile (written to DRAM), and the
    transposed scores ST = kT.T @ qT per 128-key chunk (consumed on-chip).
  - softmax skips the row-max subtraction (scores are O(6) so exp is safe in
    fp32) and gets row sums for free by augmenting v with a ones column:
    P @ [v | 1] yields both the PV product and the softmax denominators.
"""

import sys

sys.path.insert(0, "/opt/trn_rl_repo")

import numpy as np

import concourse.bacc as bacc
import concourse.tile as tile
from concourse import mybir
from concourse.bass import ds
from concourse.bass_utils import run_bass_kernel_spmd
from concourse.masks import make_identity

F32 = mybir.dt.float32
F32R = mybir.dt.float32r
AF = mybir.ActivationFunctionType
P = 128


def build_attention_nc(SQ=2048, SK=4096, H=1024, D=64):
    """Build the per-core SPMD program.

    Inputs : xq [SQ, H], xk [SK, H], xv [SK, H], wq/wk/wv [H, D], bq/bk/bv [D]
             (wq/bq arrive pre-scaled by 1/sqrt(D) so scores = qT.T @ kT).
    Outputs: scores_p [SQ, SK], out_p [SQ, D].
    """
    KO = H // P  # h chunks of 128
    NQT = SQ // P  # query tiles
    NKC = SK // P  # key chunks
    GQ = SQ // 512  # query groups (4 tiles each)

    nc = bacc.Bacc(target_bir_lowering=False, trn_type="TRN2")

    xq = nc.declare_dram_parameter("xq", [SQ, H], F32, isOutput=False)
    xk = nc.declare_dram_parameter("xk", [SK, H], F32, isOutput=False)
    xv = nc.declare_dram_parameter("xv", [SK, H], F32, isOutput=False)
    wq = nc.declare_dram_parameter("wq", [H, D], F32, isOutput=False)
    wk = nc.declare_dram_parameter("wk", [H, D], F32, isOutput=False)
    wv = nc.declare_dram_parameter("wv", [H, D], F32, isOutput=False)
    bq = nc.declare_dram_parameter("bq", [D], F32, isOutput=False)
    bk = nc.declare_dram_parameter("bk", [D], F32, isOutput=False)
    bv = nc.declare_dram_parameter("bv", [D], F32, isOutput=False)
    scores = nc.declare_dram_parameter("scores_p", [SQ, SK], F32, isOutput=True)
    outp = nc.declare_dram_parameter("out_p", [SQ, D], F32, isOutput=True)

    with tile.TileContext(nc) as tc:
        with (
            tc.tile_pool(name="consts", bufs=1) as consts,
            tc.tile_pool(name="xsp", bufs=2) as xsp,
            tc.tile_pool(name="xtp", bufs=2) as xtp,
            tc.tile_pool(name="projp", bufs=1) as projp,
            tc.tile_pool(name="sp", bufs=2) as sp,
            tc.tile_pool(name="ptp", bufs=4) as ptp,
            tc.tile_pool(name="op", bufs=2) as op,
            tc.tile_pool(name="ps_big", bufs=2, space="PSUM") as ps_big,
            tc.tile_pool(name="ps_st", bufs=2, space="PSUM") as ps_st,
            tc.tile_pool(name="ps_acc", bufs=2, space="PSUM") as ps_acc,
        ):
            ident = consts.tile([P, P], F32, name="ident")
            make_identity(nc, ident)

            w_sb = {}
            b_sb = {}
            for nm, wdram, bdram in (("q", wq, bq), ("k", wk, bk), ("v", wv, bv)):
                w_t = consts.tile([P, KO, D], F32R, name="w_sb", tag=f"w_{nm}")
                nc.sync.dma_start(
                    w_t, wdram.rearrange("(ko p) d -> p ko d", p=P).bitcast(F32R)
                )
                b_t = consts.tile([D, 1], F32, name="b_sb", tag=f"b_{nm}")
                nc.sync.dma_start(b_t, bdram[:, None])
                w_sb[nm] = w_t
                b_sb[nm] = b_t

            kT = projp.tile([D, SK], F32R, name="kT")
            qT = projp.tile([D, SQ], F32R, name="qT")
            vT = projp.tile([D, SK], F32R, name="vT")
            v_aug = projp.tile([P, NKC, D + 1], F32R, name="v_aug")
            ones32 = consts.tile([P, 1], F32, name="ones32")
            nc.vector.memset(ones32, 1.0)
            nc.vector.tensor_copy(v_aug[:, :, D], ones32.to_broadcast((P, NKC)))

            def project(xdram, s_rows, w_t, b_t, dstT):
                """dstT[:, s] = w.T @ x[s].T + b, streaming 512 rows at a time."""
                for ch in range(s_rows // 512):
                    xs = xsp.tile([P, 4, H], F32, tag="xs")
                    nc.sync.dma_start(
                        xs, xdram[ds(ch * 512, 512)].rearrange("(so p) h -> p so h", p=P)
                    )
                    xt = xtp.tile([P, KO, 512], F32R, tag="xt")
                    for kop in range(KO // 2):
                        pst = ps_big.tile([P, 1024], F32, tag="big")
                        for i in range(2):
                            ko = kop * 2 + i
                            for so in range(4):
                                nc.tensor.transpose(
                                    pst[:, ds(i * 512 + so * P, P)],
                                    xs[:, so, ds(ko * P, P)],
                                    ident,
                                )
                        dst = xt[:, ds(kop * 2, 2), :]
                        src = pst.rearrange("p (a b) -> p a b", a=2)
                        if kop % 2 == 0:
                            nc.scalar.activation(dst, src, AF.Copy)
                        else:
                            nc.vector.tensor_copy(dst, src)
                    pp = ps_acc.tile([P, 512], F32, tag="acc")
                    for ko in range(KO):
                        nc.tensor.matmul(
                            pp[:D],
                            w_t[:, ko, :],
                            xt[:, ko, :],
                            start=(ko == 0),
                            stop=(ko == KO - 1),
                        )
                    nc.scalar.activation(
                        dstT[:, ds(ch * 512, 512)], pp[:D], AF.Identity, bias=b_t
                    )

            project(xk, SK, w_sb["k"], b_sb["k"], kT)
            project(xq, SQ, w_sb["q"], b_sb["q"], qT)
            project(xv, SK, w_sb["v"], b_sb["v"], vT)

            # v_aug[:, c, :D] = vT[:, c*128:(c+1)*128].T
            for c in range(NKC):
                psv = ps_st.tile([P, 512], F32, tag="st")
                nc.tensor.transpose(psv[:, :D], vT.bitcast(F32)[:, ds(c * P, P)], ident[:D, :D])
                nc.vector.tensor_copy(v_aug[:, c, :D], psv[:, :D])

            for g in range(GQ):
                sq0 = g * 512
                # scores tiles [128, SK] -> DRAM
                for t in range(4):
                    sbig = sp.tile([P, SK], F32, tag="sbig")
                    for nk2 in range(SK // 1024):
                        ps2 = ps_big.tile([P, 1024], F32, tag="big")
                        for i in range(2):
                            nc.tensor.matmul(
                                ps2[:, ds(i * 512, 512)],
                                qT[:, ds(sq0 + t * P, P)],
                                kT[:, ds(nk2 * 1024 + i * 512, 512)],
                                start=True,
                                stop=True,
                            )
                        nc.vector.tensor_copy(sbig[:, ds(nk2 * 1024, 1024)], ps2)
                    nc.sync.dma_start(scores[ds(sq0 + t * P, P), :], sbig)

                # ST chunks -> exp -> PV accumulation (with ones column for sums)
                pv = ps_acc.tile([P, 512], F32, tag="acc")
                for c in range(NKC):
                    stp = ps_st.tile([P, 512], F32, tag="st")
                    nc.tensor.matmul(
                        stp,
                        kT[:, ds(c * P, P)],
                        qT[:, ds(sq0, 512)],
                        start=True,
                        stop=True,
                    )
                    pt = ptp.tile([P, 512], F32R, tag="pt")
                    nc.scalar.activation(pt, stp, AF.Exp)
                    nc.tensor.matmul(
                        pv[: D + 1],
                        v_aug[:, c, :],
                        pt,
                        start=(c == 0),
                        stop=(c == NKC - 1),
                    )

                # transpose unnormalized [out | l] to [sq, D+1], then divide by
                # the per-partition (per-query) sums l
                pvs = op.tile([D + 1, 512], F32, tag="pvs")
                nc.vector.tensor_copy(pvs, pv[: D + 1])
                osb = op.tile([P, 4, D], F32, tag="osb")
                for t in range(4):
                    pso = ps_st.tile([P, 512], F32, tag="st")
                    nc.tensor.transpose(
                        pso[:, : D + 1], pvs[:, ds(t * P, P)], ident[: D + 1, : D + 1]
                    )
                    rec = op.tile([P, 1], F32, tag="rec")
                    nc.vector.reciprocal(rec, pso[:, D : D + 1])
                    nc.vector.tensor_scalar_mul(osb[:, t, :], pso[:, :D], rec)
                nc.sync.dma_start(
                    outp[ds(sq0, 512)].rearrange("(so p) d -> p so d", p=P), osb
                )

    nc.finalize()
    return nc


_NC_CACHE = {}


def _get_nc(SQ, SK, H, D):
    key = (SQ, SK, H, D)
    if key not in _NC_CACHE:
        _NC_CACHE[key] = build_attention_nc(SQ, SK, H, D)
    return _NC_CACHE[key]


def kernel(x_q, x_k, x_v, Wq, bq, Wk, bk, Wv, bv):
    x_q = np.asarray(x_q, dtype=np.float32)
    x_k = np.asarray(x_k, dtype=np.float32)
    x_v = np.asarray(x_v, dtype=np.float32)
    Wq = np.asarray(Wq, dtype=np.float32)
    Wk = np.asarray(Wk, dtype=np.float32)
    Wv = np.asarray(Wv, dtype=np.float32)
    bq = np.asarray(bq, dtype=np.float32)
    bk = np.asarray(bk, dtype=np.float32)
    bv = np.asarray(bv, dtype=np.float32)

    B, S, H = x_q.shape
    D = Wq.shape[1]
    n_cores = 8
    halves = n_cores // B  # 2 query-halves per batch
    SQ = S // halves
    nc = _get_nc(SQ, S, H, D)

    # fold the 1/sqrt(D) scores scale into the q projection
    scale = np.float32(1.0) / np.sqrt(np.float32(D))
    wq_s = np.ascontiguousarray(Wq * scale)
    bq_s = np.ascontiguousarray(bq * scale)

    in_maps = []
    for core in range(n_cores):
        b, h = core // halves, core % halves
        in_maps.append(
            {
                "xq": np.ascontiguousarray(x_q[b, h * SQ : (h + 1) * SQ]),
                "xk": np.ascontiguousarray(x_k[b]),
                "xv": np.ascontiguousarray(x_v[b]),
                "wq": wq_s,
                "wk": np.ascontiguousarray(Wk),
                "wv": np.ascontiguousarray(Wv),
                "bq": bq_s,
                "bk": np.ascontiguousarray(bk),
                "bv": np.ascontiguousarray(bv),
            }
        )

    res = run_bass_kernel_spmd(nc, in_maps, core_ids=list(range(n_cores)))
    global LAST_RESULTS
    LAST_RESULTS = res

    out = np.empty((B, S, D), dtype=np.float32)
    scores = np.empty((B, S, S), dtype=np.float32)
    for core in range(n_cores):
        b, h = core // halves, core % halves
        r = res.results[core]
        out[b, h * SQ : (h + 1) * SQ] = r["out_p"]
        scores[b, h * SQ : (h + 1) * SQ] = r["scores_p"]
    return out, scores


# revision 7
# speedup vs baseline: 1.3207x; 1.3207x over previous
"""Single-head attention (q/k/v projections + scores + softmax + PV) on 8 TRN2
NeuronCores.

Sharding: core c handles batch b = c // 2 and query-half h = c % 2, i.e. 2048
queries against that batch's full 4096 keys.  Each core computes its own K/V
projections from the full x_k/x_v of its batch (duplicated across the pair of
cores sharing a batch).

Device-side layout strategy (per core):
  - x tiles are DMA'd naturally ([s, h] rows on partitions) and transposed
    on the PE (fp32 transpose-mode matmul with an identity) so the hidden dim
    lands on partitions for the projection matmuls.
  - Projections are computed directly in transposed form qT/kT/vT [64, S]
    (d on partitions) which is exactly the operand layout the scores matmuls
    need.  All matmuls use float32r (reduced-precision fp32 multiply path,
    full fp32 accumulate): ~1.6e-4 max rel error, 4x faster than true fp32.
  - scores S = qT.T @ kT per 128-query tile (written to DRAM), and the
    transposed scores ST = kT.T @ qT per 128-key chunk (consumed on-chip).
  - softmax skips the row-max subtraction (scores are O(6) so exp is safe in
    fp32) and gets row sums for free by augmenting v with a ones column:
    P @ [v | 1] yields both the PV product and the softmax denominators.
"""

import sys

sys.path.insert(0, "/opt/trn_rl_repo")

import numpy as np

import concourse.bacc as bacc
import concourse.tile as tile
from concourse import mybir
from concourse.bass import ds
from concourse.bass_utils import run_bass_kernel_spmd
from concourse.masks import make_identity

F32 = mybir.dt.float32
F32R = mybir.dt.float32r
AF = mybir.ActivationFunctionType
P = 128


def build_attention_nc(SQ=2048, SK=4096, H=1024, D=64):
    """Build the per-core SPMD program.

    Inputs : xq [SQ, H], xk [SK, H], xv [SK, H], wq/wk/wv [H, D], bq/bk/bv [D]
             (wq/bq arrive pre-scaled by 1/sqrt(D) so scores = qT.T @ kT).
    Outputs: scores_p [SQ, SK], out_p [SQ, D].
    """
    KO = H // P  # h chunks of 128
    NQT = SQ // P  # query tiles
    NKC = SK // P  # key chunks
    GQ = SQ // 512  # query groups (4 tiles each)

    nc = bacc.Bacc(target_bir_lowering=False, trn_type="TRN2")

    xq = nc.declare_dram_parameter("xq", [SQ, H], F32, isOutput=False)
    xk = nc.declare_dram_parameter("xk", [SK, H], F32, isOutput=False)
    xv = nc.declare_dram_parameter("xv", [SK, H], F32, isOutput=False)
    wq = nc.declare_dram_parameter("wq", [H, D], F32, isOutput=False)
    wk = nc.declare_dram_parameter("wk", [H, D], F32, isOutput=False)
    wv = nc.declare_dram_parameter("wv", [H, D], F32, isOutput=False)
    bq = nc.declare_dram_parameter("bq", [D], F32, isOutput=False)
    bk = nc.declare_dram_parameter("bk", [D], F32, isOutput=False)
    bv = nc.declare_dram_parameter("bv", [D], F32, isOutput=False)
    scores = nc.declare_dram_parameter("scores_p", [SQ, SK], F32, isOutput=True)
    outp = nc.declare_dram_parameter("out_p", [SQ, D], F32, isOutput=True)

    with tile.TileContext(nc) as tc:
        with (
            tc.tile_pool(name="consts", bufs=1) as consts,
            tc.tile_pool(name="xsp", bufs=2) as xsp,
            tc.tile_pool(name="xtp", bufs=2) as xtp,
            tc.tile_pool(name="projp", bufs=1) as projp,
            tc.tile_pool(name="sp", bufs=2) as sp,
            tc.tile_pool(name="ptp", bufs=4) as ptp,
            tc.tile_pool(name="op", bufs=2) as op,
            tc.tile_pool(name="ps_big", bufs=2, space="PSUM") as ps_big,
            tc.tile_pool(name="ps_st", bufs=2, space="PSUM") as ps_st,
            tc.tile_pool(name="ps_acc", bufs=2, space="PSUM") as ps_acc,
        ):
            ident = consts.tile([P, P], F32, name="ident")
            make_identity(nc, ident)

            w_sb = {}
            b_sb = {}
            for nm, wdram, bdram in (("q", wq, bq), ("k", wk, bk), ("v", wv, bv)):
                w_t = consts.tile([P, KO, D], F32R, name="w_sb", tag=f"w_{nm}")
                nc.sync.dma_start(
                    w_t, wdram.rearrange("(ko p) d -> p ko d", p=P).bitcast(F32R)
                )
                b_t = consts.tile([D, 1], F32, name="b_sb", tag=f"b_{nm}")
                nc.sync.dma_start(b_t, bdram[:, None])
                w_sb[nm] = w_t
                b_sb[nm] = b_t

            # qT/kT are padded to 128 partitions (rows D..127 zero) so the
            # scores matmuls contract over K=128: full-array activity keeps
            # the PE HAM clock-gate warm (K=64 contractions read as half-idle
            # and the PE gets stuck at 1.2 GHz).
            kT = projp.tile([P, SK], F32R, name="kT")
            qT = projp.tile([P, SQ], F32R, name="qT")
            nc.vector.memset(kT.bitcast(F32)[D:P, :], 0.0)
            nc.vector.memset(qT.bitcast(F32)[D:P, :], 0.0)
            vT = projp.tile([D, SK], F32R, name="vT")
            v_aug = projp.tile([P, NKC, D + 1], F32R, name="v_aug")
            ones32 = consts.tile([P, 1], F32, name="ones32")
            nc.vector.memset(ones32, 1.0)
            nc.vector.tensor_copy(v_aug[:, :, D], ones32.to_broadcast((P, NKC)))

            def project(xdram, s_rows, w_t, b_t, dstT):
                """dstT[:, s] = w.T @ x[s].T + b, streaming 512 rows at a time."""
                for ch in range(s_rows // 512):
                    xs = xsp.tile([P, 4, H], F32, tag="xs")
                    nc.sync.dma_start(
                        xs, xdram[ds(ch * 512, 512)].rearrange("(so p) h -> p so h", p=P)
                    )
                    xt = xtp.tile([P, KO, 512], F32R, tag="xt")
                    for kop in range(KO // 2):
                        pst = ps_big.tile([P, 1024], F32, tag="big")
                        for i in range(2):
                            ko = kop * 2 + i
                            for so in range(4):
                                nc.tensor.transpose(
                                    pst[:, ds(i * 512 + so * P, P)],
                                    xs[:, so, ds(ko * P, P)],
                                    ident,
                                )
                        dst = xt[:, ds(kop * 2, 2), :]
                        src = pst.rearrange("p (a b) -> p a b", a=2)
                        if kop % 2 == 0:
                            nc.scalar.activation(dst, src, AF.Copy)
                        else:
                            nc.vector.tensor_copy(dst, src)
                    pp = ps_acc.tile([P, 512], F32, tag="acc")
                    for ko in range(KO):
                        nc.tensor.matmul(
                            pp[:D],
                            w_t[:, ko, :],
                            xt[:, ko, :],
                            start=(ko == 0),
                            stop=(ko == KO - 1),
                        )
                    nc.scalar.activation(
                        dstT[:D, ds(ch * 512, 512)], pp[:D], AF.Identity, bias=b_t
                    )

            project(xk, SK, w_sb["k"], b_sb["k"], kT)
            project(xq, SQ, w_sb["q"], b_sb["q"], qT)
            project(xv, SK, w_sb["v"], b_sb["v"], vT)

            # v_aug[:, c, :D] = vT[:, c*128:(c+1)*128].T
            for c in range(NKC):
                psv = ps_st.tile([P, 512], F32, tag="st")
                nc.tensor.transpose(psv[:, :D], vT.bitcast(F32)[:, ds(c * P, P)], ident[:D, :D])
                nc.vector.tensor_copy(v_aug[:, c, :D], psv[:, :D])

            for g in range(GQ):
                sq0 = g * 512
                # scores tiles [128, SK] -> DRAM
                for t in range(4):
                    sbig = sp.tile([P, SK], F32, tag="sbig")
                    for nk2 in range(SK // 1024):
                        ps2 = ps_big.tile([P, 1024], F32, tag="big")
                        for i in range(2):
                            nc.tensor.matmul(
                                ps2[:, ds(i * 512, 512)],
                                qT[:, ds(sq0 + t * P, P)],
                                kT[:, ds(nk2 * 1024 + i * 512, 512)],
                                start=True,
                                stop=True,
                            )
                        nc.vector.tensor_copy(sbig[:, ds(nk2 * 1024, 1024)], ps2)
                    nc.sync.dma_start(scores[ds(sq0 + t * P, P), :], sbig)

                # ST chunks -> exp -> PV accumulation (with ones column for sums)
                pv = ps_acc.tile([P, 512], F32, tag="acc")
                for c in range(NKC):
                    stp = ps_st.tile([P, 512], F32, tag="st")
                    nc.tensor.matmul(
                        stp,
                        kT[:, ds(c * P, P)],
                        qT[:, ds(sq0, 512)],
                        start=True,
                        stop=True,
                    )
                    pt = ptp.tile([P, 512], F32R, tag="pt")
                    nc.scalar.activation(pt, stp, AF.Exp)
                    nc.tensor.matmul(
                        pv[: D + 1],
                        v_aug[:, c, :],
                        pt,
                        start=(c == 0),
                        stop=(c == NKC - 1),
                    )

                # transpose unnormalized [out | l] to [sq, D+1], then divide by
                # the per-partition (per-query) sums l
                pvs = op.tile([D + 1, 512], F32, tag="pvs")
                nc.vector.tensor_copy(pvs, pv[: D + 1])
                osb = op.tile([P, 4, D], F32, tag="osb")
                for t in range(4):
                    pso = ps_st.tile([P, 512], F32, tag="st")
                    nc.tensor.transpose(
                        pso[:, : D + 1], pvs[:, ds(t * P, P)], ident[: D + 1, : D + 1]
                    )
                    rec = op.tile([P, 1], F32, tag="rec")
                    nc.vector.reciprocal(rec, pso[:, D : D + 1])
                    nc.vector.tensor_scalar_mul(osb[:, t, :], pso[:, :D], rec)
                nc.sync.dma_start(
                    outp[ds(sq0, 512)].rearrange("(so p) d -> p so d", p=P), osb
                )

    nc.finalize()
    return nc


_NC_CACHE = {}


def _get_nc(SQ, SK, H, D):
    key = (SQ, SK, H, D)
    if key not in _NC_CACHE:
        _NC_CACHE[key] = build_attention_nc(SQ, SK, H, D)
    return _NC_CACHE[key]


def kernel(x_q, x_k, x_v, Wq, bq, Wk, bk, Wv, bv):
    x_q = np.asarray(x_q, dtype=np.float32)
    x_k = np.asarray(x_k, dtype=np.float32)
    x_v = np.asarray(x_v, dtype=np.float32)
    Wq = np.asarray(Wq, dtype=np.float32)
    Wk = np.asarray(Wk, dtype=np.float32)
    Wv = np.asarray(Wv, dtype=np.float32)
    bq = np.asarray(bq, dtype=np.float32)
    bk = np.asarray(bk, dtype=np.float32)
    bv = np.asarray(bv, dtype=np.float32)

    B, S, H = x_q.shape
    D = Wq.shape[1]
    n_cores = 8
    halves = n_cores // B  # 2 query-halves per batch
    SQ = S // halves
    nc = _get_nc(SQ, S, H, D)

    # fold the 1/sqrt(D) scores scale into the q projection
    scale = np.float32(1.0) / np.sqrt(np.float32(D))
    wq_s = np.ascontiguousarray(Wq * scale)
    bq_s = np.ascontiguousarray(bq * scale)

    in_maps = []
    for core in range(n_cores):
        b, h = core // halves, core % halves
        in_maps.append(
            {
                "xq": np.ascontiguousarray(x_q[b, h * SQ : (h + 1) * SQ]),
                "xk": np.ascontiguousarray(x_k[b]),
                "xv": np.ascontiguousarray(x_v[b]),
                "wq": wq_s,
                "wk": np.ascontiguousarray(Wk),
                "wv": np.ascontiguousarray(Wv),
                "bq": bq_s,
                "bk": np.ascontiguousarray(bk),
                "bv": np.ascontiguousarray(bv),
            }
        )

    res = run_bass_kernel_spmd(nc, in_maps, core_ids=list(range(n_cores)))
    global LAST_RESULTS
    LAST_RESULTS = res

    out = np.empty((B, S, D), dtype=np.float32)
    scores = np.empty((B, S, S), dtype=np.float32)
    for core in range(n_cores):
        b, h = core // halves, core % halves
        r = res.results[core]
        out[b, h * SQ : (h + 1) * SQ] = r["out_p"]
        scores[b, h * SQ : (h + 1) * SQ] = r["scores_p"]
    return out, scores


# revision 14
# speedup vs baseline: 1.4535x; 1.1005x over previous
"""Single-head attention (q/k/v projections + scores + softmax + PV) on 8 TRN2
NeuronCores.

Sharding: core c handles batch b = c // 2 and query-half h = c % 2, i.e. 2048
queries against that batch's full 4096 keys.  Each core computes its own K/V
projections from the full x_k/x_v of its batch (duplicated across the pair of
cores sharing a batch).

Device-side layout strategy (per core):
  - x tiles are DMA'd naturally ([s, h] rows on partitions) and transposed
    on the PE (fp32 transpose-mode matmul with an identity) so the hidden dim
    lands on partitions for the projection matmuls.
  - Projections are computed directly in transposed form qT/kT/vT [64, S]
    (d on partitions) which is exactly the operand layout the scores matmuls
    need.  All matmuls use float32r (reduced-precision fp32 multiply path,
    full fp32 accumulate): ~1.6e-4 max rel error, 4x faster than true fp32.
  - scores S = qT.T @ kT per 128-query tile (written to DRAM), and the
    transposed scores ST = kT.T @ qT per 128-key chunk (consumed on-chip).
  - softmax skips the row-max subtraction (scores are O(6) so exp is safe in
    fp32) and gets row sums for free by augmenting v with a ones column:
    P @ [v | 1] yields both the PV product and the softmax denominators.
"""

import sys

sys.path.insert(0, "/opt/trn_rl_repo")

import numpy as np

import concourse.bacc as bacc
import concourse.tile as tile
from concourse import mybir
from concourse.bass import ds
from concourse.bass_utils import run_bass_kernel_spmd
from concourse.masks import make_identity

F32 = mybir.dt.float32
F32R = mybir.dt.float32r
AF = mybir.ActivationFunctionType
P = 128


def build_attention_nc(SQ=2048, SK=4096, H=1024, D=64, n_cores=8, split_kv=True):
    """Build the per-core SPMD program.

    Inputs : xq [SQ, H], xk/xv [SKH, H], wq/wk/wv [H, D], bq/bk/bv [D]
             (wq/bq arrive pre-scaled by 1/sqrt(D) so scores = qT.T @ kT).
    Outputs: scores_p [SQ, SK], out_p [SQ, D].

    With split_kv, each core of a batch pair projects only its half of the
    keys/values (SKH = SK/2 rows of xk/xv) and the pair exchanges kT/vT
    halves via a 2-core AllGather — halving the k/v DMA + transpose work.
    """
    KO = H // P  # h chunks of 128
    NKC = SK // P  # key chunks
    GQ = SQ // 512  # query groups (4 tiles each)
    SKH = SK // 2 if split_kv else SK  # xk/xv rows this core projects

    nc = bacc.Bacc(target_bir_lowering=False, trn_type="TRN2", num_devices=n_cores)

    xq = nc.declare_dram_parameter("xq", [SQ, H], F32, isOutput=False)
    xk = nc.declare_dram_parameter("xk", [SKH, H], F32, isOutput=False)
    xv = nc.declare_dram_parameter("xv", [SKH, H], F32, isOutput=False)
    wq = nc.declare_dram_parameter("wq", [H, D], F32, isOutput=False)
    wk = nc.declare_dram_parameter("wk", [H, D], F32, isOutput=False)
    wv = nc.declare_dram_parameter("wv", [H, D], F32, isOutput=False)
    bq = nc.declare_dram_parameter("bq", [D], F32, isOutput=False)
    bk = nc.declare_dram_parameter("bk", [D], F32, isOutput=False)
    bv = nc.declare_dram_parameter("bv", [D], F32, isOutput=False)
    scores = nc.declare_dram_parameter("scores_p", [SQ, SK], F32, isOutput=True)
    outp = nc.declare_dram_parameter("out_p", [SQ, D], F32, isOutput=True)



    with tile.TileContext(nc) as tc:
        with (
            tc.tile_pool(name="consts", bufs=1) as consts,
            tc.tile_pool(name="xsp", bufs=2) as xsp,
            tc.tile_pool(name="xtp", bufs=2) as xtp,
            tc.tile_pool(name="projp", bufs=1) as projp,
            tc.tile_pool(name="sp", bufs=2) as sp,
            tc.tile_pool(name="ptp", bufs=4) as ptp,
            tc.tile_pool(name="op", bufs=2) as op,
            tc.tile_pool(name="ps_big", bufs=2, space="PSUM") as ps_big,
            tc.tile_pool(name="ps_st", bufs=2, space="PSUM") as ps_st,
            tc.tile_pool(name="ps_acc", bufs=2, space="PSUM") as ps_acc,
            tc.tile_pool(name="dramp", bufs=1, space="DRAM") as dramp,
        ):
            ident = consts.tile([P, P], F32, name="ident")
            make_identity(nc, ident)

            w_sb = {}
            b_sb = {}
            for nm, wdram, bdram in (("q", wq, bq), ("k", wk, bk), ("v", wv, bv)):
                w_t = consts.tile([P, KO, D], F32R, name="w_sb", tag=f"w_{nm}")
                nc.sync.dma_start(
                    w_t, wdram.rearrange("(ko p) d -> p ko d", p=P).bitcast(F32R)
                )
                b_t = consts.tile([D, 1], F32, name="b_sb", tag=f"b_{nm}")
                nc.sync.dma_start(b_t, bdram[:, None])
                w_sb[nm] = w_t
                b_sb[nm] = b_t

            # qT/kT are padded to 128 partitions (rows D..127 zero) so the
            # scores matmuls contract over K=128: full-array activity keeps
            # the PE HAM clock-gate warm (K=64 contractions read as half-idle
            # and the PE gets stuck at 1.2 GHz).
            kT = projp.tile([P, SK], F32R, name="kT")
            qT = projp.tile([P, SQ], F32R, name="qT")
            nc.vector.memset(kT.bitcast(F32)[D:P, :], 0.0)
            nc.vector.memset(qT.bitcast(F32)[D:P, :], 0.0)
            vT = projp.tile([D, SK], F32R, name="vT")
            v_aug = projp.tile([P, NKC, D + 1], F32R, name="v_aug")
            ones32 = consts.tile([P, 1], F32, name="ones32")
            nc.vector.memset(ones32, 1.0)
            nc.vector.tensor_copy(v_aug[:, :, D], ones32.to_broadcast((P, NKC)))

            def project(xdram, s_rows, w_t, b_t, dstT):
                """dstT[:, s] = w.T @ x[s].T + b, streaming 512 rows at a time."""
                for ch in range(s_rows // 512):
                    xs = xsp.tile([P, 4, H], F32, tag="xs")
                    nc.sync.dma_start(
                        xs, xdram[ds(ch * 512, 512)].rearrange("(so p) h -> p so h", p=P)
                    )
                    xt = xtp.tile([P, KO, 512], F32R, tag="xt")
                    for kop in range(KO // 2):
                        pst = ps_big.tile([P, 1024], F32, tag="big")
                        for i in range(2):
                            ko = kop * 2 + i
                            for so in range(4):
                                nc.tensor.transpose(
                                    pst[:, ds(i * 512 + so * P, P)],
                                    xs[:, so, ds(ko * P, P)],
                                    ident,
                                )
                        dst = xt[:, ds(kop * 2, 2), :]
                        src = pst.rearrange("p (a b) -> p a b", a=2)
                        if kop % 2 == 0:
                            nc.scalar.activation(dst, src, AF.Copy)
                        else:
                            nc.vector.tensor_copy(dst, src)
                    pp = ps_acc.tile([P, 512], F32, tag="acc")
                    for ko in range(KO):
                        nc.tensor.matmul(
                            pp[:D],
                            w_t[:, ko, :],
                            xt[:, ko, :],
                            start=(ko == 0),
                            stop=(ko == KO - 1),
                        )
                    nc.scalar.activation(
                        dstT[:D, ds(ch * 512, 512)], pp[:D], AF.Identity, bias=b_t
                    )

            if split_kv:
                # project own k/v half, exchange halves with the pair core
                kTh = projp.tile([D, SKH], F32R, name="kTh")
                vTh = projp.tile([D, SKH], F32R, name="vTh")
                project(xk, SKH, w_sb["k"], b_sb["k"], kTh)
                project(xv, SKH, w_sb["v"], b_sb["v"], vTh)
                cc_in = dramp.tile([2, D, SKH], F32, name="cc_in")
                cc_out = dramp.tile([2, 2, D, SKH], F32, name="cc_out")
                nc.sync.dma_start(cc_in[0], kTh.bitcast(F32))
                nc.sync.dma_start(cc_in[1], vTh.bitcast(F32))
                groups = [[2 * i, 2 * i + 1] for i in range(n_cores // 2)]
                nc.gpsimd.collective_compute(
                    "AllGather",
                    mybir.AluOpType.bypass,
                    replica_groups=groups,
                    ins=[cc_in[:]],
                    outs=[cc_out[:]],
                )
                project(xq, SQ, w_sb["q"], b_sb["q"], qT)
                for hf in range(2):
                    nc.sync.dma_start(
                        kT[:D, ds(hf * SKH, SKH)], cc_out[hf, 0].bitcast(F32R)
                    )
                    nc.sync.dma_start(
                        vT[:D, ds(hf * SKH, SKH)], cc_out[hf, 1].bitcast(F32R)
                    )
            else:
                project(xk, SK, w_sb["k"], b_sb["k"], kT)
                project(xq, SQ, w_sb["q"], b_sb["q"], qT)
                project(xv, SK, w_sb["v"], b_sb["v"], vT)

            # v_aug[:, c, :D] = vT[:, c*128:(c+1)*128].T
            for c in range(NKC):
                psv = ps_st.tile([P, 512], F32, tag="st")
                nc.tensor.transpose(psv[:, :D], vT.bitcast(F32)[:, ds(c * P, P)], ident[:D, :D])
                nc.vector.tensor_copy(v_aug[:, c, :D], psv[:, :D])

            for g in range(GQ):
                sq0 = g * 512
                # scores tiles [128, SK] -> DRAM
                for t in range(4):
                    sbig = sp.tile([P, SK], F32, tag="sbig")
                    for nk2 in range(SK // 1024):
                        ps2 = ps_big.tile([P, 1024], F32, tag="big")
                        for i in range(2):
                            nc.tensor.matmul(
                                ps2[:, ds(i * 512, 512)],
                                qT[:, ds(sq0 + t * P, P)],
                                kT[:, ds(nk2 * 1024 + i * 512, 512)],
                                start=True,
                                stop=True,
                            )
                        nc.vector.tensor_copy(sbig[:, ds(nk2 * 1024, 1024)], ps2)
                    nc.sync.dma_start(scores[ds(sq0 + t * P, P), :], sbig)

                # ST chunks -> exp -> PV accumulation (with ones column for sums)
                pv = ps_acc.tile([P, 512], F32, tag="acc")
                for c in range(NKC):
                    stp = ps_st.tile([P, 512], F32, tag="st")
                    nc.tensor.matmul(
                        stp,
                        kT[:, ds(c * P, P)],
                        qT[:, ds(sq0, 512)],
                        start=True,
                        stop=True,
                    )
                    pt = ptp.tile([P, 512], F32R, tag="pt")
                    nc.scalar.activation(pt, stp, AF.Exp)
                    nc.tensor.matmul(
                        pv[: D + 1],
                        v_aug[:, c, :],
                        pt,
                        start=(c == 0),
                        stop=(c == NKC - 1),
                    )

                # transpose unnormalized [out | l] to [sq, D+1], then divide by
                # the per-partition (per-query) sums l
                pvs = op.tile([D + 1, 512], F32, tag="pvs")
                nc.vector.tensor_copy(pvs, pv[: D + 1])
                osb = op.tile([P, 4, D], F32, tag="osb")
                for t in range(4):
                    pso = ps_st.tile([P, 512], F32, tag="st")
                    nc.tensor.transpose(
                        pso[:, : D + 1], pvs[:, ds(t * P, P)], ident[: D + 1, : D + 1]
                    )
                    rec = op.tile([P, 1], F32, tag="rec")
                    nc.vector.reciprocal(rec, pso[:, D : D + 1])
                    nc.vector.tensor_scalar_mul(osb[:, t, :], pso[:, :D], rec)
                nc.sync.dma_start(
                    outp[ds(sq0, 512)].rearrange("(so p) d -> p so d", p=P), osb
                )

    nc.finalize()
    return nc


_NC_CACHE = {}


def _get_nc(SQ, SK, H, D):
    key = (SQ, SK, H, D)
    if key not in _NC_CACHE:
        _NC_CACHE[key] = build_attention_nc(SQ, SK, H, D)
    return _NC_CACHE[key]


def kernel(x_q, x_k, x_v, Wq, bq, Wk, bk, Wv, bv):
    x_q = np.asarray(x_q, dtype=np.float32)
    x_k = np.asarray(x_k, dtype=np.float32)
    x_v = np.asarray(x_v, dtype=np.float32)
    Wq = np.asarray(Wq, dtype=np.float32)
    Wk = np.asarray(Wk, dtype=np.float32)
    Wv = np.asarray(Wv, dtype=np.float32)
    bq = np.asarray(bq, dtype=np.float32)
    bk = np.asarray(bk, dtype=np.float32)
    bv = np.asarray(bv, dtype=np.float32)

    B, S, H = x_q.shape
    D = Wq.shape[1]
    n_cores = 8
    halves = n_cores // B  # 2 query-halves per batch
    SQ = S // halves
    nc = _get_nc(SQ, S, H, D)

    # fold the 1/sqrt(D) scores scale into the q projection
    scale = np.float32(1.0) / np.sqrt(np.float32(D))
    wq_s = np.ascontiguousarray(Wq * scale)
    bq_s = np.ascontiguousarray(bq * scale)

    SKH = S // halves
    in_maps = []
    for core in range(n_cores):
        b, h = core // halves, core % halves
        in_maps.append(
            {
                "xq": np.ascontiguousarray(x_q[b, h * SQ : (h + 1) * SQ]),
                "xk": np.ascontiguousarray(x_k[b, h * SKH : (h + 1) * SKH]),
                "xv": np.ascontiguousarray(x_v[b, h * SKH : (h + 1) * SKH]),
                "wq": wq_s,
                "wk": np.ascontiguousarray(Wk),
                "wv": np.ascontiguousarray(Wv),
                "bq": bq_s,
                "bk": np.ascontiguousarray(bk),
                "bv": np.ascontiguousarray(bv),
            }
        )

    res = run_bass_kernel_spmd(nc, in_maps, core_ids=list(range(n_cores)))
    global LAST_RESULTS
    LAST_RESULTS = res

    out = np.empty((B, S, D), dtype=np.float32)
    scores = np.empty((B, S, S), dtype=np.float32)
    for core in range(n_cores):
        b, h = core // halves, core % halves
        r = res.results[core]
        out[b, h * SQ : (h + 1) * SQ] = r["out_p"]
        scores[b, h * SQ : (h + 1) * SQ] = r["scores_p"]
    return out, scores
